# revision 24
# baseline (speedup 1.0000x reference)
"""Trainium2 Bass kernel for cosine-sim multi-head attention.

Model (per batch element):
    xn  = l2norm(x) * g * sqrt(D)
    qkv = xn @ w_qkv ; split q,k,v ; heads of 64
    q   = l2norm(q) * q_scale ; k = l2norm(k) * k_scale
    out = softmax(8 * q k^T) v ; merge heads ; @ w_out

Sharding: data-parallel over batch (B=8) across the 8 NeuronCores.  Each
core runs an identical single-core program on its own batch element; no
collectives.

Key facts used:
  - the RMSNorm row scale cancels inside l2norm(q), l2norm(k); it only
    affects v.  q,k are computed from *raw* x (g folded into w_qkv rows),
    and the row scale sqrt(D)/||x|| is applied to v only.
  - q_scale*k_scale folds into ktil per-partition (d-dim layout).
  - all 1/sqrt() use a quake-style bit-trick seed + one Newton step on
    the otherwise-idle GPSIMD/DVE engines (the seed's exponent halving
    is done by converting the bit pattern through f32 since neither
    engine has integer shifts).  The ACT engine therefore runs ONLY the
    softmax Exp (plus a few prologue copies) - exactly one activation
    table load, no exp/sqrt table thrash.
  - 1/||k_j|| (and the 8x logit scale) are applied via the per-partition
    ACT scale of the softmax exp: S^T has k-tokens on partitions.
    k-token sumsq is computed in token-partition layout with 2-column
    [128x128]^T @ block-ones matmuls.
  - 1/||q_i|| must be materialized along the free dim of S^T, so it is
    computed in the d-partition layout: per-head sumsq replication via a
    block-diagonal ones matmul, then rsqrt, then one DVE multiply.
  - attention runs in two phases per (head, i-half) block: phase A
    streams 16 S^T j-tiles through PSUM into exp, holding the E tiles in
    SBUF; phase B accumulates attn@V per 128-token i-tile with E slices
    as lhsT, so the output lands [i-part, v(64)|den(1)] using all 128 PE
    output partitions, one accumulation group per PSUM bank (a hardware
    limit), and the denominator normalizes via a per-partition DVE
    scalar multiply - no DMA round trips.
  - prep work (QKV projections, norms) is emitted as closures
    interleaved into earlier attention j-loops, software-pipelining the
    engines' static in-order schedules; matmul accumulation groups keep
    a constant operand base partition (alternating bases hangs the PE).
  - scores are bounded (|S| <= ~8.2), softmax needs no max-subtraction.
  - everything on the PE runs in bf16 (same 1 cycle/row as f32r, half
    the SBUF) except the x transposes (f32r).
"""

import math
import os
import sys

import numpy as np

sys.path.insert(0, "/opt/trn_rl_repo")

N = 2048
D = 512
H = 8
DH = 64
P = 128
NT = N // P  # 16 token tiles
DC = D // P  # 4 contraction chunks
SCALE = 8.0
NCORES = 8


def build_attention(nc, out_ap, x_ap, g_ap, wqkv_ap, qs_ap, ks_ap, wout_ap):
    """Emit the full single-core attention program into `nc` (Tile)."""
    import concourse.mybir as mybir
    from concourse.masks import make_identity
    from concourse.tile import TileContext

    f32 = mybir.dt.float32
    bf16 = mybir.dt.bfloat16
    f32r = mybir.dt.float32r

    def R(ap):
        return ap.bitcast(f32r)

    AF = mybir.ActivationFunctionType
    OP = mybir.AluOpType
    AX = mybir.AxisListType
    u32 = mybir.dt.uint32

    def rsqrt_pool(dst, src, a, pool, tag, eng2=None, eng=None):
        """dst = a / sqrt(src), elementwise: quake bit-trick seed + one
        Newton step.  src/dst SBUF f32.  ~1.8e-3 max rel err, scaled by
        exact constant a.

        Neither DVE nor Pool has integer shifts, so the seed's exponent
        halving is done in the float domain: convert bits(s) to f32 (24-bit
        mantissa rounding adds only ~1.5e-5, far below the seed error),
        apply MAGIC - bits/2 as an fma, convert back to u32, bitcast."""
        E = eng or nc.gpsimd
        E2 = eng2 or E
        y = pool.tile(list(src.shape), f32, tag=f"{tag}y")
        t = pool.tile(list(src.shape), f32, tag=f"{tag}t")
        E.tensor_copy(t, src.bitcast(u32))  # u32 -> f32 (numeric convert)
        E.tensor_scalar(t, t, -0.5, float(0x5F3759DF), OP.mult, OP.add)
        E.tensor_copy(y.bitcast(u32), t)    # f32 -> u32 (numeric convert)
        # one Newton step: y1 = y0 * (1.5a - 0.5a*s*y0^2)/a, folded with *a
        E2.tensor_tensor(t, y, y, OP.mult)
        E2.tensor_tensor(t, t, src, OP.mult)
        E2.tensor_scalar(t, t, -0.5 * a, 1.5 * a, OP.mult, OP.add)
        E2.tensor_tensor(dst, y, t, OP.mult)

    with TileContext(nc) as tc:
        # ---------------- persistent pools --------------------------------
        const = tc.alloc_tile_pool(name="const", bufs=1)
        big = tc.alloc_tile_pool(name="big", bufs=1)
        dscr = tc.alloc_tile_pool(name="dscr", bufs=1, space="DRAM")

        ident = const.tile([P, P], f32)
        make_identity(nc, ident)
        identb = const.tile([P, P], bf16)
        make_identity(nc, identb)
        # block-diagonal ones (bf16): per-head partition-sum matmul
        # replicates each head's sumsq across that head's 64 partitions
        ind2 = const.tile([P, P], bf16)
        nc.vector.memset(ind2, 0.0)
        nc.vector.memset(ind2[0:DH, 0:DH], 1.0)
        nc.vector.memset(ind2[DH:P, DH:P], 1.0)
        # two-column per-head-parity ones: col0 selects partitions 0:64,
        # col1 selects 64:128 - one matmul yields both heads' sumsq
        ones2 = const.tile([P, 2], bf16)
        nc.vector.memset(ones2, 0.0)
        nc.vector.memset(ones2[0:DH, 0:1], 1.0)
        nc.vector.memset(ones2[DH:P, 1:2], 1.0)

        # q_scale * k_scale, replicated to 128 partitions via DRAM bounce
        qs_sb = const.tile([DH, 1], f32)
        ks_sb = const.tile([DH, 1], f32)
        nc.sync.dma_start(qs_sb, qs_ap[:, None])
        nc.sync.dma_start(ks_sb, ks_ap[:, None])
        qsks64 = const.tile([DH, 1], f32)
        nc.vector.tensor_tensor(qsks64, qs_sb, ks_sb, OP.mult)
        qsks_d = dscr.tile([DH], f32, tag="qsks")
        nc.sync.dma_start(qsks_d[:, None], qsks64)
        qsks = const.tile([P, 1], f32)
        nc.sync.dma_start(qsks[0:DH, :], qsks_d[:, None])
        nc.sync.dma_start(qsks[DH:P, :], qsks_d[:, None])

        # persistent big tensors (all bf16)
        xT = big.tile([P, DC, N], bf16)        # x^T, [d-part(c), token]
        wqb = big.tile([P, DC, 3 * D], bf16)   # w_qkv * g, [d-part(c), outcol]
        qhat = big.tile([P, DC, N], bf16)      # q-hat^T, chunk m = heads 2m,2m+1
        ktil = big.tile([P, DC, N], bf16)      # k^T * qsks (NOT length-normed)
        V1 = big.tile([P, NT, H, DH + 1], bf16)  # per head [srow*v_h(64) | 1]
        woh = big.tile([DH, H, D], bf16)       # w_out rows per head
        attnT = big.tile([DH, H, N], bf16)     # per head out^T [d, token]
        srow = const.tile([P, NT], f32)        # sqrt(D)/||x_row||
        krecip = const.tile([P, DC, 2 * NT], f32)  # 8/||k_j||, [j-part, pr, hp*16+jt]

        # ---------------- per-head-pair prep + attention ------------------
        psP = tc.alloc_tile_pool(name="psP", bufs=2, space="PSUM")
        sqp = tc.alloc_tile_pool(name="sqp", bufs=3)
        qrp = tc.alloc_tile_pool(name="qrp", bufs=2)
        stn = tc.alloc_tile_pool(name="stn", bufs=3)
        drec = tc.alloc_tile_pool(name="drec", bufs=16, space="DRAM")

        def prep_pieces(pr):
            """QK projection, norms, V1 for heads (2pr, 2pr+1), returned as
            a list of closures.  The caller interleaves them into the
            previous pair's attention j-loops so the engines' static
            in-order schedules naturally software-pipeline.

            Q side first (its rsqrt chains are the longest pole), per-ih
            granularity everywhere so attention j-tiles unblock as early
            as possible.  For pr==0 the PSUM->SBUF copies ride the still-
            idle ACT engine (Copy shares the exp table set)."""
            cp = nc.scalar.copy if pr == 0 else nc.vector.tensor_copy
            pieces = []

            # Q chunk: q outcols pr*128 .. +128 -> qhat[:, pr, :]
            qsq = sqp.tile([P, N], bf16, tag="sq", name=f"qsq{pr}")

            def q_proj(ih):
                pm = psP.tile([P, D], f32, tag="p", name=f"qp{pr}_{ih}")
                for c in range(DC):
                    nc.tensor.matmul(
                        pm,
                        lhsT=wqb[:, c, pr * P : (pr + 1) * P],
                        rhs=xT[:, c, ih * D : (ih + 1) * D],
                        start=(c == 0),
                        stop=(c == DC - 1),
                    )
                cp(qhat[:, pr, ih * D : (ih + 1) * D], pm)

            def q_norm(ih):
                nc.vector.tensor_tensor(
                    qsq[:, ih * D : (ih + 1) * D],
                    qhat[:, pr, ih * D : (ih + 1) * D],
                    qhat[:, pr, ih * D : (ih + 1) * D],
                    OP.mult,
                )
                pq = psP.tile([P, D], f32, tag="p", name=f"pq{pr}_{ih}")
                nc.tensor.matmul(
                    pq,
                    lhsT=ind2,
                    rhs=qsq[:, ih * D : (ih + 1) * D],
                    start=True,
                    stop=True,
                )
                sq_sb = qrp.tile([P, D], f32, tag="sqsb", name=f"sqsb{pr}_{ih}")
                cp(sq_sb, pq)
                qrec = qrp.tile([P, D], f32, tag="qrec", name=f"qrec{pr}_{ih}")
                rsqrt_pool(
                    qrec, sq_sb, 1.0, qrp, "qr",
                    eng2=nc.vector, eng=nc.vector,
                )
                nc.vector.tensor_tensor(
                    qhat[:, pr, ih * D : (ih + 1) * D],
                    qhat[:, pr, ih * D : (ih + 1) * D],
                    qrec,
                    OP.mult,
                )

            for ih in range(4):
                pieces.append(lambda ih=ih: q_proj(ih))
                pieces.append(lambda ih=ih: q_norm(ih))

            # K chunk: k outcols 512 + pr*128 .. +128 -> ktil[:, pr, :]
            ksq = sqp.tile([P, N], bf16, tag="sq", name=f"ksq{pr}")
            pk = psP.tile([P, 2 * NT], f32, tag="p", name=f"pk{pr}")
            sk = stn.tile([P, 2 * NT], f32, tag="sk", name=f"sk{pr}")

            def k_piece(ih):
                pm = psP.tile([P, D], f32, tag="p", name=f"kp{pr}_{ih}")
                for c in range(DC):
                    nc.tensor.matmul(
                        pm,
                        lhsT=wqb[:, c, D + pr * P : D + (pr + 1) * P],
                        rhs=xT[:, c, ih * D : (ih + 1) * D],
                        start=(c == 0),
                        stop=(c == DC - 1),
                    )
                cp(ktil[:, pr, ih * D : (ih + 1) * D], pm)
                nc.vector.tensor_tensor(
                    ksq[:, ih * D : (ih + 1) * D],
                    ktil[:, pr, ih * D : (ih + 1) * D],
                    ktil[:, pr, ih * D : (ih + 1) * D],
                    OP.mult,
                )
                for jt in range(ih * 4, ih * 4 + 4):
                    nc.tensor.matmul(
                        pk[:, jt * 2 : jt * 2 + 2],
                        lhsT=ksq[:, jt * P : (jt + 1) * P],
                        rhs=ones2,
                        start=True,
                        stop=True,
                    )
                # qsks fold per-ih, right after its ksq is taken
                nc.vector.tensor_scalar_mul(
                    ktil[:, pr, ih * D : (ih + 1) * D],
                    ktil[:, pr, ih * D : (ih + 1) * D],
                    qsks,
                )
                if ih == 1:
                    cp(sk[:, 0:NT], pk[:, 0:NT])
                    rsqrt_pool(krecip[:, pr, 0:NT], sk[:, 0:NT], SCALE, stn,
                               "kr", eng2=nc.vector)
                elif ih == 3:
                    cp(sk[:, NT : 2 * NT], pk[:, NT : 2 * NT])
                    rsqrt_pool(krecip[:, pr, NT : 2 * NT], sk[:, NT : 2 * NT],
                               SCALE, stn, "kr", eng2=nc.vector)

            for ih in range(4):
                pieces.append(lambda ih=ih: k_piece(ih))

            # V for heads 2pr, 2pr+1: v outcols 1024 + pr*128 .. +128
            def v_piece(t):
                if t == 0:
                    nc.vector.memset(
                        V1[:, :, 2 * pr : 2 * pr + 2, DH : DH + 1], 1.0
                    )
                pv = psP.tile([P, D], f32, tag="p", name=f"pv{pr}_{t}")
                for c in range(DC):
                    nc.tensor.matmul(
                        pv[:, 0:P],
                        lhsT=xT[:, c, t * P : (t + 1) * P],
                        rhs=wqb[:, c, 2 * D + pr * P : 2 * D + (pr + 1) * P],
                        start=(c == 0),
                        stop=(c == DC - 1),
                    )
                nc.vector.tensor_scalar_mul(
                    V1[:, t, 2 * pr : 2 * pr + 2, 0:DH],
                    pv[:, 0:P].rearrange("p (h c) -> p h c", c=DH),
                    srow[:, t : t + 1],
                )

            for t in range(NT):
                pieces.append(lambda t=t: v_piece(t))
            return pieces

        # prep0 pieces, annotated with the last x tile each needs so the
        # prologue can interleave them into the transpose stream.
        _stage0 = int(os.environ.get("KERNEL_STAGE", "99"))
        _p0 = prep_pieces(0) if _stage0 >= 0 else []
        # order: [qp0,qn0,qp1,qn1,qp2,qn2,qp3,qn3, k0,k1,k2,k3, v0..v15]
        prep_pieces_grouped0 = []
        for q in range(4 if _stage0 >= 0 else 0):
            prep_pieces_grouped0.append((4 * q + 3, _p0[2 * q]))      # q_proj
            prep_pieces_grouped0.append((4 * q + 3, _p0[2 * q + 1]))  # q_norm
            prep_pieces_grouped0.append((4 * q + 3, _p0[8 + q]))      # k_piece
            for v in range(4):
                t = 4 * q + v
                # v_piece(t) needs xT tile t AND srow (half-chains emitted
                # at t==7 / t==15)
                prep_pieces_grouped0.append((max(t, 7 if t < 8 else 15),
                                             _p0[12 + t]))

        # ---------------- prologue: loads, transposes, srow ---------------
        with tc.tile_pool(name="xnat", bufs=4) as xnp, \
             tc.tile_pool(name="wld", bufs=2) as wld, \
             tc.tile_pool(name="st1", bufs=2) as st1, \
             tc.tile_pool(name="ps_tr", bufs=2, space="PSUM") as ps_tr:
            # DMA order matters: the DMA device is bandwidth-bound, so load
            # exactly what unblocks the first K/Q projections first - the
            # leading x tiles and w_qkv - then stream the rest of x.
            # x tiles rotate through 6 slots: each is consumed (row-norm +
            # transpose) well before its slot is needed again.
            x_nat = [
                xnp.tile([P, D], f32, tag="xn", name=f"xn{t}") for t in range(NT)
            ]
            g_sb = st1.tile([P, DC], f32, tag="g")
            xre = x_ap.rearrange("(t p) d -> p t d", p=P)
            ss_x = const.tile([P, NT], f32)
            hnt = NT // 2

            def load_x(t):
                nc.sync.dma_start(x_nat[t], xre[:, t, :])
                xsq = st1.tile([P, D], f32, tag="xsq")
                nc.vector.tensor_tensor(xsq, x_nat[t], x_nat[t], OP.mult)
                nc.vector.tensor_reduce(
                    ss_x[:, t : t + 1], xsq, AX.X, OP.add
                )

            for t in range(4):
                load_x(t)
            nc.sync.dma_start(g_sb, g_ap.rearrange("(c p) -> p c", p=P))
            for c in range(DC):
                wf = wld.tile([P, 3 * D], f32, tag="wld", name=f"wld{c}")
                nc.sync.dma_start(
                    R(wf), R(wqkv_ap.rearrange("(c p) q -> p c q", p=P)[:, c, :])
                )
                nc.scalar.activation(
                    wqb[:, c, :], wf, AF.Copy, scale=g_sb[:, c : c + 1]
                )

            # transpose x into xT (bf16): 4 chunks per psum bank, 1 copy.
            # prep0 pieces are fed in as soon as their xT ranges exist, so
            # the first attention block is not gated on the full x load.
            def transpose_tile(t):
                pst = ps_tr.tile([P, D], f32, tag="tr")
                for c in range(DC):
                    nc.tensor.transpose(
                        pst[:, c * P : (c + 1) * P],
                        x_nat[t][:, c * P : (c + 1) * P],
                        ident,
                    )
                (nc.scalar.copy if t < 8 else nc.vector.tensor_copy)(
                    xT[:, :, t * P : (t + 1) * P],
                    pst.rearrange("p (c q) -> p c q", c=DC),
                )

            # interleave prep0 pieces gated on the xT ranges they read
            pf = prep_pieces_grouped0
            for t in range(NT):
                if t + 4 < NT:
                    load_x(t + 4)
                transpose_tile(t)
                if t == hnt - 1:
                    rsqrt_pool(srow[:, 0:hnt], ss_x[:, 0:hnt],
                               math.sqrt(float(D)), st1, "sr")
                elif t == NT - 1:
                    rsqrt_pool(srow[:, hnt:NT], ss_x[:, hnt:NT],
                               math.sqrt(float(D)), st1, "sr")
                while pf and pf[0][0] <= t:
                    pf.pop(0)[1]()
            # w_out after x (not needed until the projection at the end)
            for hh in range(0, H, 2):
                wo_f = wld.tile([DH, 2, D], f32, tag="wo", name=f"wo{hh}")
                wre = wout_ap.rearrange("(h p) o -> p h o", p=DH)[:, hh : hh + 2, :]
                nc.sync.dma_start(R(wo_f), R(wre))
                nc.vector.tensor_copy(woh[:, hh : hh + 2, :], wo_f)
            while pf:
                pf.pop(0)[1]()

        ep = tc.alloc_tile_pool(name="ep", bufs=18)
        sto = tc.alloc_tile_pool(name="sto", bufs=2)
        psS = tc.alloc_tile_pool(name="psS", bufs=2, space="PSUM")
        psA = tc.alloc_tile_pool(name="psA", bufs=2, space="PSUM")

        DH1 = DH + 1

        def attention(pr, half, hp, feed=None):
            """softmax(8 qk^T) v for head h=2pr+hp, i-range half.

            Phase A streams S^T j-tiles through PSUM into exp (per-partition
            scale applies 8/||k_j||), holding all 16 E tiles in SBUF.
            Phase B then accumulates attn@V per 128-token i-tile with E
            slices as lhsT: output lands [i-part, v(64)|den(1)] using all
            128 PE output partitions, one accumulation group per PSUM bank.
            The denominator column normalizes via a per-partition DVE
            scalar multiply; results stash until the partner head's, then
            one PE transpose writes both heads' [d, i] slabs."""
            h = 2 * pr + hp
            lo = hp * DH
            ioff = half * (N // 2)
            Es = []
            for j in range(NT):
                ps = psS.tile([P, 2 * D], f32, tag="s", name=f"s{pr}_{half}_{hp}_{j}")
                for ii in range(2):
                    nc.tensor.matmul(
                        ps[:, ii * D : (ii + 1) * D],
                        lhsT=ktil[lo : lo + DH, pr, j * P : (j + 1) * P],
                        rhs=qhat[lo : lo + DH, pr, ioff + ii * D : ioff + (ii + 1) * D],
                        start=True,
                        stop=True,
                    )
                E = ep.tile([P, 2 * D], bf16, tag="e", name=f"e{pr}_{half}_{hp}_{j}")
                nc.scalar.activation(
                    E, ps, AF.Exp, scale=krecip[:, pr, j * 2 + hp : j * 2 + hp + 1]
                )
                Es.append(E)
                if feed:
                    feed.pop(0)()
            def b_accum():
                # j-outer: both i-half accumulators live in their own
                # banks; E(j) is dead right after its two matmuls (the
                # shared stationary V1 also halves the weight loads), so
                # the next block's exp pipeline restarts without waiting
                # for this whole phase.
                accs = [
                    psA.tile([DH1, D], f32, tag="a",
                             name=f"acc{pr}_{half}_{hp}_{i}")
                    for i in range(2)
                ]
                for j in range(NT):
                    for ii in range(2):
                        nc.tensor.matmul(
                            accs[ii],
                            lhsT=V1[:, j, h, :],
                            rhs=Es[j][:, ii * D : (ii + 1) * D],
                            start=(j == 0),
                            stop=(j == NT - 1),
                        )
                return accs

            def b_norm(accs, ii):
                stg = stn.tile([DH1, D], f32, tag="stg",
                               name=f"stg{pr}_{half}_{hp}_{ii}")
                nc.vector.tensor_copy(stg, accs[ii])
                rd = drec.tile([D], f32, tag="recd")
                nc.sync.dma_start(rd[None, :], stg[DH : DH1, :])
                recb = stn.tile([DH, D], f32, tag="recb",
                                name=f"recb{pr}_{half}_{hp}_{ii}")
                nc.sync.dma_start(recb, rd[None, :].to_broadcast([DH, D]))
                nc.vector.reciprocal_approx_fast(out=recb, in_=recb)
                nc.vector.tensor_tensor(
                    attnT[:, h, ioff + ii * D : ioff + (ii + 1) * D],
                    stg[0:DH, :],
                    recb,
                    OP.mult,
                )

            holder = []

            def b_all():
                accs = b_accum()
                b_norm(accs, 0)
                b_norm(accs, 1)

            out_pieces = [b_all]
            return out_pieces

        def proj_piece(t):
            """output projection for one 128-token tile (all operands at
            base partition 0 - constant base per accumulation group)."""
            po = psP.tile([P, D], f32, tag="p", name=f"po{t}")
            for h in range(H):
                nc.tensor.matmul(
                    po,
                    lhsT=attnT[:, h, t * P : (t + 1) * P],
                    rhs=woh[:, h, :],
                    start=(h == 0),
                    stop=(h == H - 1),
                )
            osb = sto.tile([P, D], f32, tag="o")
            nc.vector.tensor_copy(osb, po)
            nc.sync.dma_start(out_ap[t * P : (t + 1) * P, :], osb)

        # software-pipelined emission: prep(pr+1) pieces and proj tiles
        # are interleaved into the attention j-loops so every engine's
        # static in-order schedule has independent work at each stall.
        stage = int(os.environ.get("KERNEL_STAGE", "99"))
        feed = []
        for pr in range(4):
            if pr >= stage:
                break
            if pr < 3:
                feed = feed + prep_pieces(pr + 1)
            for half in range(2):
                if pr == 3 and half == 1 and stage > 4:
                    feed = feed + [
                        (lambda t=t: proj_piece(t)) for t in range(NT // 2)
                    ]
                with nc.named_scope(f"attn{pr}{half}"):
                    for piece in attention(pr, half, 0, feed):
                        piece()
                    for piece in attention(pr, half, 1, feed):
                        piece()
        with nc.named_scope("proj1"):
            for piece in feed:
                piece()
            if stage > 4:
                for t in range(NT // 2, NT):
                    proj_piece(t)

        for pool in (psA, psS, sto, ep, drec, stn, qrp, sqp, psP, dscr, big, const):
            pool.release()


def _build_nc():
    import concourse.mybir as mybir
    from concourse import bacc

    f32 = mybir.dt.float32
    nc = bacc.Bacc("TRN2", target_bir_lowering=False, debug=False)
    x = nc.dram_tensor("x", [N, D], f32, kind="ExternalInput")
    g = nc.dram_tensor("g", [D], f32, kind="ExternalInput")
    w_qkv = nc.dram_tensor("w_qkv", [D, 3 * D], f32, kind="ExternalInput")
    q_scale = nc.dram_tensor("q_scale", [DH], f32, kind="ExternalInput")
    k_scale = nc.dram_tensor("k_scale", [DH], f32, kind="ExternalInput")
    w_out = nc.dram_tensor("w_out", [D, D], f32, kind="ExternalInput")
    out = nc.dram_tensor("out", [N, D], f32, kind="ExternalOutput")
    build_attention(
        nc, out[:], x[:], g[:], w_qkv[:], q_scale[:], k_scale[:], w_out[:]
    )
    nc.finalize()
    return nc


def _bench_spmd(nc, in_maps, n_cores, iters=48, warmup=8):
    """Steady-state device-time estimate: replicate run_bass_via_pjrt's
    jit, pre-stage inputs + donated zero buffers, time K pipelined calls
    and report the per-iteration slope.  NOTE: on axon-tunneled setups
    each call carries ~2.2-2.4 ms of dispatch overhead that is NOT
    pipelined away; the printed number includes it."""
    import time

    import jax
    import numpy as np_
    from jax.sharding import Mesh, PartitionSpec
    from jax.experimental.shard_map import shard_map

    from concourse import bass2jax
    from concourse import mybir

    bass2jax.install_neuronx_cc_hook()
    partition_name = nc.partition_id_tensor.name if nc.partition_id_tensor else None
    in_names, out_names, out_avals, zero_outs = [], [], [], []
    for alloc in nc.m.functions[0].allocations:
        if not isinstance(alloc, mybir.MemoryLocationSet):
            continue
        name = alloc.memorylocations[0].name
        if alloc.kind == "ExternalInput":
            if name != partition_name:
                in_names.append(name)
        elif alloc.kind == "ExternalOutput":
            shape = tuple(alloc.tensor_shape)
            dt = mybir.dt.np(alloc.dtype)
            out_names.append(name)
            out_avals.append(jax.core.ShapedArray(shape, dt))
            zero_outs.append(np_.zeros(shape, dt))
    n_params = len(in_names)
    n_outs = len(out_avals)
    in_names = in_names + out_names
    if partition_name is not None:
        in_names.append(partition_name)
    donate = tuple(range(n_params, n_params + n_outs))

    def _body(*args):
        operands = list(args)
        if partition_name is not None:
            operands.append(bass2jax.partition_id_tensor())
        outs = bass2jax._bass_exec_p.bind(
            *operands,
            out_avals=tuple(out_avals),
            in_names=tuple(in_names),
            out_names=tuple(out_names),
            lowering_input_output_aliases=(),
            sim_require_finite=True,
            sim_require_nnan=True,
            nc=nc,
        )
        return tuple(outs)

    devices = jax.devices()[:n_cores]
    mesh = Mesh(np_.asarray(devices), ("core",))
    sharded = jax.jit(
        shard_map(
            _body,
            mesh=mesh,
            in_specs=(PartitionSpec("core"),) * (n_params + n_outs),
            out_specs=(PartitionSpec("core"),) * len(out_names),
            check_rep=False,
        ),
        donate_argnums=donate,
        keep_unused=True,
    )
    per_core = [[np_.asarray(m[name]) for name in in_names[:n_params]] for m in in_maps]
    concat_in = [
        np_.concatenate([per_core[c][i] for c in range(n_cores)], axis=0)
        for i in range(n_params)
    ]
    sh = jax.sharding.NamedSharding(mesh, PartitionSpec("core"))
    dev_in = [jax.device_put(a, sh) for a in concat_in]

    def zeros_set():
        return [
            jax.device_put(np_.zeros((n_cores * z.shape[0], *z.shape[1:]), z.dtype), sh)
            for z in zero_outs
        ]

    total = warmup + iters
    zsets = [zeros_set() for _ in range(total)]
    for z in zsets:
        jax.block_until_ready(z)
    outs = []
    for i in range(warmup):
        outs = sharded(*dev_in, *zsets[i])
    jax.block_until_ready(outs)
    t0 = time.perf_counter()
    for i in range(warmup, total):
        outs = sharded(*dev_in, *zsets[i])
    jax.block_until_ready(outs)
    t1 = time.perf_counter()
    per_iter_ns = (t1 - t0) / iters * 1e9
    return per_iter_ns


def kernel(x, g, w_qkv, q_scale, k_scale, w_out):
    from concourse.bass_utils import run_bass_kernel_spmd

    nc = _build_nc()
    x = np.ascontiguousarray(np.asarray(x, dtype=np.float32))
    shared = {
        "g": np.asarray(g, np.float32),
        "w_qkv": np.ascontiguousarray(np.asarray(w_qkv, np.float32)),
        "q_scale": np.asarray(q_scale, np.float32),
        "k_scale": np.asarray(k_scale, np.float32),
        "w_out": np.ascontiguousarray(np.asarray(w_out, np.float32)),
    }
    in_maps = [{"x": x[i], **shared} for i in range(NCORES)]
    trace = bool(int(os.environ.get("KERNEL_TRACE", "0")))
    try:
        res = run_bass_kernel_spmd(
            nc, in_maps, core_ids=list(range(NCORES)), trace=trace
        )
    except ModuleNotFoundError:
        # NTFF profile hook unavailable in this container; run untraced.
        res = run_bass_kernel_spmd(
            nc, in_maps, core_ids=list(range(NCORES)), trace=False
        )
    if res.exec_time_ns is not None:
        print(f"HW exec time: {res.exec_time_ns} ns")
        if res.instructions_and_trace is not None:
            print(f"trace path: {res.instructions_and_trace[1]}")
        if res.profile_json is not None:
            print(f"profile json: {res.profile_json}")
        if res.per_core_scope_times:
            for scope, times in sorted(res.per_core_scope_times.items()):
                print(f"  scope {scope}: {times}")
    elif int(os.environ.get("KERNEL_BENCH", "0")):
        per_iter = _bench_spmd(nc, in_maps, NCORES)
        print(f"HW exec time: {per_iter:.0f} ns")
    out = np.stack([res.results[i]["out"] for i in range(NCORES)], axis=0)
    return out.astype(np.float32)

